# revision 1
# baseline (speedup 1.0000x reference)
"""Trainium2 Bass kernel for nn_MiddleLayerEncoder (gnn_message_passing).

Strategy: shard by CLUSTER across the 8 cores (each core owns 512 whole
clusters and all of their points), so both segment_max reductions are
core-local and no collectives are needed.  Host-side prep sorts points by
cluster, pads every cluster to a canonical per-rank size (identical across
cores -> single SPMD program), and bakes all segment boundaries into the
instruction stream.  Activations are kept transposed [feat, points] so the
per-cluster max is a free-axis windowed reduce; the neigh_enc[cluster]
gather is fused into the W1 matmul via a per-tile local one-hot operand.
"""

import numpy as np
import ml_dtypes
from contextlib import ExitStack

import concourse.bass as bass
import concourse.bacc as bacc
import concourse.tile as tile
from concourse import mybir
from concourse.bass_utils import run_bass_kernel_spmd

BF16 = mybir.dt.bfloat16
F32 = mybir.dt.float32
NPBF16 = ml_dtypes.bfloat16

N_CORES = 8
N_PTS = 262144
N_CLUSTERS = 4096
G = 24            # one-hot rows (max clusters overlapping a 512-col tile)
K1 = 3 + 64 + G   # layer-1 contraction: pts(3) + feat(64) + onehot(G)
MINL = 36         # minimum padded points per cluster (bounds clusters/tile)
CHUNK_COLS = 8192
TILE = 512


# ---------------------------------------------------------------- planning

def _plan(cluster):
    """Canonical SPMD layout shared by all cores."""
    counts = np.bincount(cluster, minlength=N_CLUSTERS)
    assert counts.min() >= 1, "empty cluster unsupported"
    order = np.argsort(-counts, kind="stable")  # cluster ids, size desc

    # snake-deal into N_CORES bins -> per-core 512 clusters, balanced sizes
    n_ranks = N_CLUSTERS // N_CORES
    cids = np.empty((N_CORES, n_ranks), dtype=np.int64)
    for i, cid in enumerate(order):
        rnd, pos = divmod(i, N_CORES)
        core = pos if rnd % 2 == 0 else N_CORES - 1 - pos
        cids[core, rnd] = cid

    sizes = counts[cids]                      # [cores, ranks]
    L = sizes.max(axis=0)                     # canonical per-rank size
    L = np.maximum((L + 3) // 4 * 4, MINL).astype(np.int64)

    col0 = np.concatenate([[0], np.cumsum(L)])  # rank -> start col
    S = int(col0[-1])

    # chunks: whole clusters, <= CHUNK_COLS cols, <= 128 clusters
    chunks = []  # (r0, r1, c0, cols)
    r0 = 0
    while r0 < n_ranks:
        r1 = r0
        while (
            r1 < n_ranks
            and (col0[r1 + 1] - col0[r0]) <= CHUNK_COLS
            and (r1 - r0) < 128
        ):
            r1 += 1
        chunks.append((r0, r1, int(col0[r0]), int(col0[r1] - col0[r0])))
        r0 = r1

    # rank of every column (canonical)
    col_rank = np.repeat(np.arange(n_ranks), L)

    # layer-1 512-col tiles per chunk: (c0, cols, base_rank, n_rank_rows)
    tiles = []
    for (r0, r1, c0, cc) in chunks:
        tl = []
        for t0 in range(0, cc, TILE):
            tc = min(TILE, cc - t0)
            base = int(col_rank[c0 + t0])
            last = int(col_rank[c0 + t0 + tc - 1])
            nrows = last - base + 1
            assert nrows <= G, f"tile spans {nrows} clusters > G={G}"
            tl.append((c0 + t0, tc, base, nrows))
        tiles.append(tl)

    # size classes (global, for stage-2 reduce): runs of equal L
    classes = []  # (rank0, n, w_points)
    i = 0
    while i < n_ranks:
        j = i
        while j < n_ranks and L[j] == L[i]:
            j += 1
        classes.append((i, j - i, int(L[i])))
        i = j

    return dict(
        cids=cids, sizes=sizes, L=L, col0=col0, S=S, chunks=chunks,
        col_rank=col_rank, tiles=tiles, classes=classes, n_ranks=n_ranks,
    )


def _prep_core(k, plan, rel_points, features, cluster, sort_idx, bucket0):
    """Per-core input arrays (canonical layout, core-specific data)."""
    L, col0, S = plan["L"], plan["col0"], plan["S"]
    cids = plan["cids"][k]
    n_ranks = plan["n_ranks"]

    slot = np.empty(S, dtype=np.int64)
    for r in range(n_ranks):
        cid = cids[r]
        idx = sort_idx[bucket0[cid]: bucket0[cid + 1]]
        n = idx.shape[0]
        c0, c1 = col0[r], col0[r + 1]
        m = c1 - c0
        slot[c0: c0 + n] = idx
        if m > n:
            slot[c0 + n: c1] = idx[0]

    pts = rel_points[slot]          # [S, 3] f32
    feat = features[slot]           # [S, 64] f32

    # encT: [K1, S] = ptsT(3) + featT(64) + onehot(G)
    encT = np.zeros((K1, S), dtype=NPBF16)
    encT[0:3] = pts.T.astype(NPBF16)
    encT[3:67] = feat.T.astype(NPBF16)
    col_rank = plan["col_rank"]
    oh_row = np.empty(S, dtype=np.int64)
    for tl in plan["tiles"]:
        for (c0, tc, base, nrows) in tl:
            oh_row[c0: c0 + tc] = col_rank[c0: c0 + tc] - base
    encT[67 + oh_row, np.arange(S)] = NPBF16(1.0)

    # pts4: [12, S/4] quad-packed points
    pts4 = (
        pts.astype(NPBF16)
        .reshape(S // 4, 4, 3)
        .transpose(1, 2, 0)
        .reshape(12, S // 4)
    )
    pts4 = np.ascontiguousarray(pts4)
    return {"encT": encT, "pts4": pts4}


def _blockdiag(w, times):
    fi, fo = w.shape
    out = np.zeros((fi * times, fo * times), dtype=w.dtype)
    for i in range(times):
        out[i * fi:(i + 1) * fi, i * fo:(i + 1) * fo] = w
    return out


def _prep_weights(inp):
    bf = lambda a: np.ascontiguousarray(a.astype(NPBF16))
    f32c = lambda a: np.ascontiguousarray(a.reshape(-1, 1).astype(np.float32))
    W1 = inp["W1"]
    return {
        "enc1_lhsT": bf(_blockdiag(inp["enc_W1"], 4)),       # [12,128]
        "b_enc1_4": f32c(np.tile(inp["enc_b1"], 4)),          # [128,1]
        "enc2_lhsT": bf(_blockdiag(inp["enc_W2"], 2)),        # [64,128]
        "b_enc2": f32c(inp["enc_b2"]),                        # [64,1]
        "W1ab": bf(W1[0:67]),                                 # [67,128]
        "W1c": bf(W1[67:131]),                                # [64,128]
        "b1": f32c(inp["b1"]),
        "fcW2": bf(inp["W2"]),                                # [128,128]
        "b2": f32c(inp["b2"]),
        "G1": bf(inp["G1"]),
        "gb1": f32c(inp["gb1"]),
        "G2a": bf(inp["G2"][:, 0:128]),
        "G2b": bf(inp["G2"][:, 128:256]),
        "gb2a": f32c(inp["gb2"][0:128]),
        "gb2b": f32c(inp["gb2"][128:256]),
    }


# ---------------------------------------------------------------- program

def _build(plan, reps=1, want_tc=False):
    S = plan["S"]
    nc = bacc.Bacc(None, target_bir_lowering=False, debug=True)

    encT_d = nc.dram_tensor("encT", [K1, S], BF16, kind="ExternalInput")
    pts4_d = nc.dram_tensor("pts4", [12, S // 4], BF16, kind="ExternalInput")
    wspec = [
        ("enc1_lhsT", [12, 128], BF16), ("b_enc1_4", [128, 1], F32),
        ("enc2_lhsT", [64, 128], BF16), ("b_enc2", [64, 1], F32),
        ("W1ab", [67, 128], BF16), ("W1c", [64, 128], BF16),
        ("b1", [128, 1], F32), ("fcW2", [128, 128], BF16),
        ("b2", [128, 1], F32), ("G1", [128, 128], BF16),
        ("gb1", [128, 1], F32), ("G2a", [128, 128], BF16),
        ("G2b", [128, 128], BF16), ("gb2a", [128, 1], F32),
        ("gb2b", [128, 1], F32),
    ]
    w_d = {n: nc.dram_tensor(n, sh, dt, kind="ExternalInput") for n, sh, dt in wspec}
    out_d = nc.dram_tensor("out", [256, 512], F32, kind="ExternalOutput")

    RELU = mybir.ActivationFunctionType.Relu
    COPY = mybir.ActivationFunctionType.Copy

    with tile.TileContext(nc) as tc, ExitStack() as ctx:
        consts = ctx.enter_context(tc.tile_pool(name="consts", bufs=1))
        glob = ctx.enter_context(tc.tile_pool(name="glob", bufs=1))
        stream = ctx.enter_context(tc.tile_pool(name="stream", bufs=3))
        stream2 = ctx.enter_context(tc.tile_pool(name="stream2", bufs=2))
        small = ctx.enter_context(tc.tile_pool(name="small", bufs=4))
        tree1 = ctx.enter_context(tc.tile_pool(name="tree1", bufs=2))
        tree2 = ctx.enter_context(tc.tile_pool(name="tree2", bufs=2))
        ps_a = ctx.enter_context(tc.tile_pool(name="ps_a", bufs=2, space="PSUM"))
        ps_p = ctx.enter_context(tc.tile_pool(name="ps_p", bufs=6, space="PSUM"))

        w_sb = {}
        for n, sh, dt in wspec:
            t = consts.tile(sh, dt, tag=f"w_{n}")
            nc.sync.dma_start(out=t[:], in_=w_d[n][:])
            w_sb[n] = t
        # copy of enc2 weights staged at partitions 64-127 (matmul requires
        # lhsT and rhs to share base_partition; the B-half rhs lives there)
        enc2_hi = consts.tile([128, 128], BF16, tag="w_enc2_hi")
        nc.sync.dma_start(out=enc2_hi[64:128, :], in_=w_d["enc2_lhsT"][:])

        Cbuf = glob.tile([128, S // 4], BF16, tag="Cbuf")
        Dbuf = glob.tile([128, S // 2], BF16, tag="Dbuf")
        pre_neigh = glob.tile([128, plan["n_ranks"]], BF16, tag="pre_neigh")
        neighT = glob.tile([64, plan["n_ranks"]], BF16, tag="neighT")
        gT = glob.tile([128, plan["n_ranks"]], BF16, tag="gT")
        T2buf = glob.tile([128, plan["n_ranks"]], BF16, tag="T2buf")

        n_chunks = len(plan["chunks"])
        ST = 512   # pair-stage super-tile width (PSUM banks)

        def enc_stage(k):
            (r0, r1, c0, cc) = plan["chunks"][k]
            q0, qc = c0 // 4, cc // 4
            pts4_t = stream.tile([12, qc], BF16, tag="pts4_t", bufs=2)
            encT_t = stream.tile([K1, cc], BF16, tag="encT_t")
            np_dma = 4 if k == 0 else 2
            for d in range(np_dma):
                a, b = qc * d // np_dma, qc * (d + 1) // np_dma
                nc.sync.dma_start(out=pts4_t[:, a:b], in_=pts4_d[:, q0 + a:q0 + b])
            for d in range(np_dma):
                a, b = cc * d // np_dma, cc * (d + 1) // np_dma
                nc.sync.dma_start(out=encT_t[:, a:b], in_=encT_d[:, c0 + a:c0 + b])
            h1_t = stream.tile([128, qc], BF16, tag="h1_t", bufs=2)
            for s0 in range(0, qc, ST):
                sc = min(ST, qc - s0)
                # enc1 + h1 evac in 512-wide sub-tiles
                for t0 in range(s0, s0 + sc, TILE):
                    tcn = min(TILE, s0 + sc - t0)
                    p1 = ps_a.tile([128, TILE], F32, tag="psa")
                    nc.tensor.matmul(p1[:, :tcn], w_sb["enc1_lhsT"][:],
                                     pts4_t[:, t0:t0 + tcn], start=True, stop=True)
                    nc.scalar.activation(h1_t[:, t0:t0 + tcn], p1[:, :tcn], RELU,
                                         bias=w_sb["b_enc1_4"][:], scale=1.0)
                # enc2 pair over the super tile
                pA = ps_p.tile([128, ST], F32, tag="psp")
                pB = ps_p.tile([128, ST], F32, tag="psp")
                for t0 in range(0, sc, TILE):
                    tcn = min(TILE, sc - t0)
                    sl = slice(s0 + t0, s0 + t0 + tcn)
                    nc.tensor.matmul(pA[:, t0:t0 + tcn], w_sb["enc2_lhsT"][:],
                                     h1_t[0:64, sl], start=True, stop=True)
                    nc.tensor.matmul(pB[:, t0:t0 + tcn], enc2_hi[64:128, :],
                                     h1_t[64:128, sl], start=True, stop=True)
                A_sb = small.tile([128, ST], BF16, tag="A1sb", bufs=6)
                nc.scalar.activation(A_sb[:, :sc], pA[:, :sc], COPY)
                nc.vector.tensor_max(Cbuf[:, q0 + s0: q0 + s0 + sc],
                                     pB[:, :sc], A_sb[:, :sc])
            return encT_t  # consumed later by l_stage(k)

        def seg1_and_M(k):
            (r0, r1, c0, cc) = plan["chunks"][k]
            q0 = c0 // 4
            nk = r1 - r0
            # per-class: one bf16 TT-halving level (2x rate), then reduce
            i = r0
            while i < r1:
                j = i
                w = int(plan["L"][i])
                while j < r1 and plan["L"][j] == w:
                    j += 1
                n, wq = j - i, w // 4
                o = q0 + (int(plan["col0"][i]) - c0) // 4
                a, odd = wq // 2, wq % 2
                wh = a + odd
                hb = tree1.tile([128, 2048], BF16, tag="tree1")
                dst = hb[:, : n * wh].rearrange("p (n w) -> p n w", w=wh)
                csrc = Cbuf[:, o: o + n * wq].rearrange("p (n w) -> p n w", w=wq)
                nc.vector.tensor_max(dst[:, :, :a], csrc[:, :, :a],
                                     csrc[:, :, a:2 * a])
                if odd:
                    nc.vector.tensor_copy(dst[:, :, a:], csrc[:, :, 2 * a:])
                nc.vector.reduce_max(pre_neigh[:, i:j],
                                     hb[:, : n * wh].rearrange(
                                         "p (n w) -> p n w", w=wh),
                                     axis=mybir.AxisListType.X)
                i = j
            fold = small.tile([64, 128], BF16, tag="fold")
            nc.sync.dma_start(out=fold[:, :nk], in_=pre_neigh[64:128, r0:r1])
            mx = small.tile([64, 128], BF16, tag="mx")
            nc.vector.tensor_max(mx[:, :nk], pre_neigh[0:64, r0:r1], fold[:, :nk])
            nc.scalar.activation(neighT[:, r0:r1], mx[:, :nk], RELU,
                                 bias=w_sb["b_enc2"][:], scale=1.0)
            pm = ps_a.tile([128, 128], F32, tag="psa")
            nc.tensor.matmul(pm[:nk, :], neighT[:, r0:r1], w_sb["W1c"][:],
                             start=True, stop=True)
            M_chunk = small.tile([128, 128], BF16, tag="Mchunk")
            if nk < 128:
                nc.vector.memset(M_chunk[:], 0.0)  # zero first, then fill
            nc.scalar.activation(M_chunk[:nk, :], pm[:nk, :], COPY)
            return M_chunk

        def l_stage(k, M_chunk):
            (r0, r1, c0, cc) = plan["chunks"][k]
            encT_t = enc_tiles[k]
            e1_t = stream2.tile([128, cc], BF16, tag="e1_t")
            e1_v = e1_t[:].rearrange("p (h i) -> p i h", h=2)
            for ti, (tc0, tcn, base, nrows) in enumerate(plan["tiles"][k]):
                lt = small.tile([128, 128], BF16, tag="lhsT1", bufs=6)
                # always copy G rows (zeros beyond this chunk's clusters come
                # from M_chunk's zeroed tail); clamp at partition 128
                gr = min(G, 128 - (base - r0))
                if gr < G:
                    nc.vector.memset(lt[64:128, :], 0.0)  # aligned, pre-fill
                nc.sync.dma_start(out=lt[0:67, :], in_=w_sb["W1ab"][:])
                nc.sync.dma_start(out=lt[67:67 + gr, :],
                                  in_=M_chunk[base - r0: base - r0 + gr, :])
                p = ps_a.tile([128, TILE], F32, tag="psa")
                loc = tc0 - c0
                nc.tensor.matmul(p[:, :tcn], lt[0:K1, :],
                                 encT_t[:, loc: loc + tcn], start=True, stop=True)
                dst = e1_v[:, loc // 2: (loc + tcn) // 2, :]
                src = p[:, :tcn].rearrange("p (i h) -> p i h", h=2)
                if ti % 3 != 0:
                    nc.scalar.activation(dst, src, RELU, bias=w_sb["b1"][:],
                                         scale=1.0)
                else:
                    nc.vector.tensor_scalar(
                        dst, src, w_sb["b1"][:], 0.0,
                        op0=mybir.AluOpType.add, op1=mybir.AluOpType.max,
                    )
            # layer 2 + stage-2 L1 pairing (1024-wide super tiles)
            d0 = c0 // 2
            half = cc // 2
            for s0 in range(0, half, ST):
                sc = min(ST, half - s0)
                pA = ps_p.tile([128, ST], F32, tag="psp")
                pB = ps_p.tile([128, ST], F32, tag="psp")
                for t0 in range(0, sc, TILE):
                    tcn = min(TILE, sc - t0)
                    nc.tensor.matmul(pA[:, t0:t0 + tcn], w_sb["fcW2"][:],
                                     e1_t[:, s0 + t0: s0 + t0 + tcn],
                                     start=True, stop=True)
                    nc.tensor.matmul(pB[:, t0:t0 + tcn], w_sb["fcW2"][:],
                                     e1_t[:, half + s0 + t0: half + s0 + t0 + tcn],
                                     start=True, stop=True)
                A_sb = small.tile([128, ST], BF16, tag="A2sb", bufs=6)
                nc.scalar.activation(A_sb[:, :sc], pA[:, :sc], COPY)
                nc.vector.tensor_max(Dbuf[:, d0 + s0: d0 + s0 + sc],
                                     pB[:, :sc], A_sb[:, :sc])
            # stage-2 per-class: TT-halving level then reduce
            T2 = T2buf
            i = r0
            while i < r1:
                j = i
                w = int(plan["L"][i])
                while j < r1 and plan["L"][j] == w:
                    j += 1
                n, wd = j - i, w // 2
                o = int(plan["col0"][i]) // 2
                a, odd = wd // 2, wd % 2
                wh = a + odd
                hb = tree2.tile([128, 3072], BF16, tag="tree2")
                dst = hb[:, : n * wh].rearrange("p (n w) -> p n w", w=wh)
                dsrc = Dbuf[:, o: o + n * wd].rearrange("p (n w) -> p n w", w=wd)
                nc.vector.tensor_max(dst[:, :, :a], dsrc[:, :, :a],
                                     dsrc[:, :, a:2 * a])
                if odd:
                    nc.vector.tensor_copy(dst[:, :, a:], dsrc[:, :, 2 * a:])
                nc.vector.reduce_max(T2[:, i:j],
                                     hb[:, : n * wh].rearrange(
                                         "p (n w) -> p n w", w=wh),
                                     axis=mybir.AxisListType.X)
                i = j

        for rep in range(reps):
            # 3-stage software pipeline: enc(k+2) | seg1M(k+1) | l(k)
            enc_tiles = {}
            M_of = {}
            enc_tiles[0] = enc_stage(0)
            M_of[0] = seg1_and_M(0)
            if n_chunks > 1:
                enc_tiles[1] = enc_stage(1)
            for k in range(n_chunks):
                if k + 2 < n_chunks:
                    enc_tiles[k + 2] = enc_stage(k + 2)
                if k + 1 < n_chunks:
                    M_of[k + 1] = seg1_and_M(k + 1)
                l_stage(k, M_of.pop(k))
                del enc_tiles[k]

            nc.scalar.activation(gT[:], T2buf[:], RELU, bias=w_sb["b2"][:],
                                 scale=1.0)

            # global MLP
            pg = ps_a.tile([128, 512], F32, tag="psa")
            nc.tensor.matmul(pg[:], w_sb["G1"][:], gT[:], start=True, stop=True)
            g1T = glob.tile([128, 512], BF16, tag="g1T")
            nc.scalar.activation(g1T[:], pg[:], RELU, bias=w_sb["gb1"][:],
                                 scale=1.0)
            for half, (wn, bn) in enumerate((("G2a", "gb2a"), ("G2b", "gb2b"))):
                po = ps_a.tile([128, 512], F32, tag="psa")
                nc.tensor.matmul(po[:], w_sb[wn][:], g1T[:], start=True,
                                 stop=True)
                o_sb = glob.tile([128, 512], F32, tag=f"osb{half}")
                nc.scalar.activation(o_sb[:], po[:], RELU, bias=w_sb[bn][:],
                                     scale=1.0)
                nc.sync.dma_start(out=out_d[half * 128:(half + 1) * 128, :],
                                  in_=o_sb[:])

    nc.finalize()
    if want_tc:
        return nc, tc
    return nc


# ---------------------------------------------------------------- entry

_CACHE = {}


def _run(inputs, trace=False, **spmd_kwargs):
    cluster = np.asarray(inputs["cluster"])
    key = hash(cluster.tobytes())
    if key not in _CACHE:
        plan = _plan(cluster)
        nc = _build(plan)
        _CACHE[key] = (plan, nc)
    plan, nc = _CACHE[key]

    rel_points = np.asarray(inputs["relative_points"], dtype=np.float32)
    features = np.asarray(inputs["features"], dtype=np.float32)
    sort_idx = np.argsort(cluster, kind="stable")
    bucket0 = np.concatenate(
        [[0], np.cumsum(np.bincount(cluster, minlength=N_CLUSTERS))]
    )
    wmap = _prep_weights({k: np.asarray(v, dtype=np.float32)
                          for k, v in inputs.items()
                          if k not in ("relative_points", "features", "cluster")})

    in_maps = []
    for k in range(N_CORES):
        m = _prep_core(k, plan, rel_points, features, cluster, sort_idx, bucket0)
        m.update(wmap)
        in_maps.append(m)

    res = run_bass_kernel_spmd(nc, in_maps, list(range(N_CORES)),
                               trace=trace, **spmd_kwargs)

    out = np.empty((N_CLUSTERS, 256), dtype=np.float32)
    for k in range(N_CORES):
        out[plan["cids"][k]] = res.results[k]["out"].T
    return out, res


def kernel(**inputs):
    return _run(inputs)[0]



# revision 10
# speedup vs baseline: 1.5141x; 1.5141x over previous
"""Trainium2 Bass kernel for nn_MiddleLayerEncoder (gnn_message_passing).

Strategy: shard by CLUSTER across the 8 cores (each core owns 512 whole
clusters and all their points), so both segment_max reductions are
core-local and no collectives are needed.

v2 layout: points are sorted by cluster, padded per-rank to a canonical
size L (multiple of 8, min 40, identical across cores -> one SPMD
program), grouped into chunks of <= 61 ranks / <= 4096 columns, and each
chunk's columns are PAIR-SPLIT: each cluster's first L/2 points go into
the chunk's first half and the rest into the second half at the same
relative offset.  Stage-2 segment_max then starts with a single
contiguous max of the two chunk halves, so every PSUM evacuation is a
contiguous wide (1024/2048-col) op.  The neigh_enc[cluster] gather is
fused into the W1 matmul via per-chunk one-hot rows (rank-within-chunk,
<= 61 rows), letting one constant lhsT serve the whole chunk.
"""

import numpy as np
import ml_dtypes
from contextlib import ExitStack

import concourse.bass as bass
import concourse.bacc as bacc
import concourse.tile as tile
from concourse import mybir
from concourse.bass_utils import run_bass_kernel_spmd

BF16 = mybir.dt.bfloat16
F32 = mybir.dt.float32
NPBF16 = ml_dtypes.bfloat16

N_CORES = 8
N_PTS = 262144
N_CLUSTERS = 4096
MINL = 40          # minimum padded points per cluster (multiple of 8)
MAX_RANKS = 61     # chunk rank cap: 67 + 61 <= 128 lhsT partitions
MAX_COLS = 4096    # chunk column cap (SBUF/PSUM tiling)

# l1 evacuation engine pattern per 2048-col window (S=scalar, V=vector)
L1_EVAC = "SSV"


# ---------------------------------------------------------------- planning

def _plan(cluster):
    """Canonical SPMD layout shared by all cores."""
    counts = np.bincount(cluster, minlength=N_CLUSTERS)
    assert counts.min() >= 1, "empty cluster unsupported"
    order = np.argsort(-counts, kind="stable")  # cluster ids, size desc

    # snake-deal into N_CORES bins -> per-core 512 clusters, balanced sizes
    n_ranks = N_CLUSTERS // N_CORES
    cids = np.empty((N_CORES, n_ranks), dtype=np.int64)
    for i, cid in enumerate(order):
        rnd, pos = divmod(i, N_CORES)
        core = pos if rnd % 2 == 0 else N_CORES - 1 - pos
        cids[core, rnd] = cid

    sizes = counts[cids]                      # [cores, ranks]
    L = sizes.max(axis=0)                     # canonical per-rank size
    L = np.maximum((L + 7) // 8 * 8, MINL).astype(np.int64)

    col0 = np.concatenate([[0], np.cumsum(L)])  # rank -> start col
    S = int(col0[-1])

    # chunks: whole clusters, <= MAX_COLS cols, <= MAX_RANKS ranks
    chunks = []  # (r0, r1, c0, cc)
    r0 = 0
    while r0 < n_ranks:
        r1 = r0
        while (
            r1 < n_ranks
            and (col0[r1 + 1] - col0[r0]) <= MAX_COLS
            and (r1 - r0) < MAX_RANKS
        ):
            r1 += 1
        chunks.append((r0, r1, int(col0[r0]), int(col0[r1] - col0[r0])))
        r0 = r1

    # per-chunk class runs (consecutive ranks with equal L) and local
    # half-offsets: hoff[r] = sum of L/2 for ranks r0..r-1
    classes = []   # per chunk: list of (i, j, hoff_i) with equal L[i:j]
    for (r0, r1, c0, cc) in chunks:
        cl = []
        i = r0
        while i < r1:
            j = i
            while j < r1 and L[j] == L[i]:
                j += 1
            hoff = int((col0[i] - col0[r0]) // 2)
            cl.append((i, j, hoff))
            i = j
        classes.append(cl)

    return dict(
        cids=cids, sizes=sizes, L=L, col0=col0, S=S, chunks=chunks,
        classes=classes, n_ranks=n_ranks,
    )


def _prep_core(k, plan, rel_points, features, cluster, sort_idx, bucket0):
    """Per-core input arrays (canonical layout, core-specific data)."""
    L, col0, S = plan["L"], plan["col0"], plan["S"]
    cids = plan["cids"][k]

    slot = np.empty(S, dtype=np.int64)
    oh_row = np.empty(S, dtype=np.int64)   # one-hot row (67 + rank-in-chunk)
    for (r0, r1, c0, cc) in plan["chunks"]:
        ch = cc // 2
        pos = 0
        for r in range(r0, r1):
            cid = cids[r]
            idx = sort_idx[bucket0[cid]: bucket0[cid + 1]]
            n = idx.shape[0]
            Lr = int(L[r])
            h = Lr // 2
            padded = np.empty(Lr, dtype=np.int64)
            padded[:n] = idx
            if Lr > n:
                padded[n:] = idx[0]
            slot[c0 + pos: c0 + pos + h] = padded[:h]
            slot[c0 + ch + pos: c0 + ch + pos + h] = padded[h:]
            oh_row[c0 + pos: c0 + pos + h] = 64 + (r - r0)
            oh_row[c0 + ch + pos: c0 + ch + pos + h] = 64 + (r - r0)
            pos += h

    pts = rel_points[slot]          # [S, 3] f32
    feat = features[slot]           # [S, 64] f32

    # rows 0:64 = pts + feat[0:61]  (matches W1[0:64])
    # rows 64:125 = one-hot (rank-in-chunk); rows 125:128 = feat[61:64]
    encT = np.zeros((128, S), dtype=NPBF16)
    encT[0:3] = pts.T.astype(NPBF16)
    encT[3:64] = feat.T[0:61].astype(NPBF16)
    encT[125:128] = feat.T[61:64].astype(NPBF16)
    encT[oh_row, np.arange(S)] = NPBF16(1.0)

    # pts4: [12, S/4] quad-packed points
    pts4 = (
        pts.astype(NPBF16)
        .reshape(S // 4, 4, 3)
        .transpose(1, 2, 0)
        .reshape(12, S // 4)
    )
    pts4 = np.ascontiguousarray(pts4)
    return {"encT": encT, "pts4": pts4}


def _blockdiag(w, times):
    fi, fo = w.shape
    out = np.zeros((fi * times, fo * times), dtype=w.dtype)
    for i in range(times):
        out[i * fi:(i + 1) * fi, i * fo:(i + 1) * fo] = w
    return out


def _prep_weights(inp):
    bf = lambda a: np.ascontiguousarray(a.astype(NPBF16))
    f32c = lambda a: np.ascontiguousarray(a.reshape(-1, 1).astype(np.float32))
    W1 = inp["W1"]
    return {
        "enc1_lhsT": bf(_blockdiag(inp["enc_W1"], 4)),       # [12,128]
        "b_enc1_4": f32c(np.tile(inp["enc_b1"], 4)),          # [128,1]
        "enc2_lhsT": bf(_blockdiag(inp["enc_W2"], 2)),        # [64,128]
        "b_enc2": f32c(inp["enc_b2"]),                        # [64,1]
        "W1A": bf(W1[0:64]),                                  # [64,128]
        "W1B": bf(W1[64:67]),                                 # [3,128]
        "W1c": bf(W1[67:131]),                                # [64,128]
        "b1": f32c(inp["b1"]),
        "fcW2": bf(inp["W2"]),                                # [128,128]
        "b2": f32c(inp["b2"]),
        "G1": bf(inp["G1"]),
        "gb1": f32c(inp["gb1"]),
        "G2a": bf(inp["G2"][:, 0:128]),
        "G2b": bf(inp["G2"][:, 128:256]),
        "gb2a": f32c(inp["gb2"][0:128]),
        "gb2b": f32c(inp["gb2"][128:256]),
    }


# ---------------------------------------------------------------- program

def _build(plan):
    S = plan["S"]
    L = plan["L"]
    n_ranks = plan["n_ranks"]
    n_chunks = len(plan["chunks"])
    nc = bacc.Bacc(None, target_bir_lowering=False, debug=True)

    encT_d = nc.dram_tensor("encT", [128, S], BF16, kind="ExternalInput")
    pts4_d = nc.dram_tensor("pts4", [12, S // 4], BF16, kind="ExternalInput")
    wspec = [
        ("enc1_lhsT", [12, 128], BF16), ("b_enc1_4", [128, 1], F32),
        ("enc2_lhsT", [64, 128], BF16), ("b_enc2", [64, 1], F32),
        ("W1A", [64, 128], BF16), ("W1B", [3, 128], BF16),
        ("W1c", [64, 128], BF16),
        ("b1", [128, 1], F32), ("fcW2", [128, 128], BF16),
        ("b2", [128, 1], F32), ("G1", [128, 128], BF16),
        ("gb1", [128, 1], F32), ("G2a", [128, 128], BF16),
        ("G2b", [128, 128], BF16), ("gb2a", [128, 1], F32),
        ("gb2b", [128, 1], F32),
    ]
    w_d = {n: nc.dram_tensor(n, sh, dt, kind="ExternalInput") for n, sh, dt in wspec}
    out_d = nc.dram_tensor("out", [256, 512], F32, kind="ExternalOutput")

    RELU = mybir.ActivationFunctionType.Relu
    COPY = mybir.ActivationFunctionType.Copy

    with tile.TileContext(nc) as tc, ExitStack() as ctx:
        consts = ctx.enter_context(tc.tile_pool(name="consts", bufs=1))
        glob = ctx.enter_context(tc.tile_pool(name="glob", bufs=1))
        stream = ctx.enter_context(tc.tile_pool(name="stream", bufs=3))
        mid = ctx.enter_context(tc.tile_pool(name="mid", bufs=2))
        small = ctx.enter_context(tc.tile_pool(name="small", bufs=3))
        ps = ctx.enter_context(tc.tile_pool(name="ps", bufs=4, space="PSUM"))

        w_sb = {}
        for n, sh, dt in wspec:
            t = consts.tile(sh, dt, tag=f"w_{n}")
            nc.sync.dma_start(out=t[:], in_=w_d[n][:])
            w_sb[n] = t
        # enc2 weights staged at partitions 64-127 (matmul requires lhsT
        # and rhs to share base_partition; the B-half rhs lives there)
        enc2_hi = consts.tile([128, 128], BF16, tag="w_enc2_hi")
        nc.sync.dma_start(out=enc2_hi[64:128, :], in_=w_d["enc2_lhsT"][:])

        pre_neigh = glob.tile([128, n_ranks], BF16, tag="pre_neigh")
        T2buf = glob.tile([128, n_ranks], BF16, tag="T2buf")
        nT = glob.tile([64, 128], BF16, tag="nT")
        nc.vector.memset(nT[:], 0.0)

        enc_tiles = {}   # chunk -> encT_t (consumed by C stage)
        cb_tiles = {}    # chunk -> Cb tile
        lhsT_of = {}     # chunk -> per-chunk l1 lhsT
        l1_widx = [0]    # global l1 window counter (evac engine pattern)

        def stage_A(k):
            """DMA + encoder MLP -> Cb (quad-col pre-max)."""
            (r0, r1, c0, cc) = plan["chunks"][k]
            cq = cc // 4
            encT_t = stream.tile([128, MAX_COLS], BF16, tag="encT_t")
            nc.sync.dma_start(out=encT_t[:, :cc], in_=encT_d[:, c0:c0 + cc])
            pts4_t = mid.tile([12, MAX_COLS // 4], BF16, tag="pts4_t")
            nc.gpsimd.dma_start(out=pts4_t[:, :cq],
                                in_=pts4_d[:, c0 // 4: c0 // 4 + cq])

            # enc1: [12 -> 128] on quad-packed points
            pe1 = ps.tile([128, 1024], F32, tag="ps")
            for s in range(0, cq, 512):
                w = min(512, cq - s)
                nc.tensor.matmul(pe1[:, s:s + w], w_sb["enc1_lhsT"][:],
                                 pts4_t[:, s:s + w], start=True, stop=True)
            h1 = mid.tile([128, 1024], BF16, tag="h1")
            nc.scalar.activation(h1[:, :cq], pe1[:, :cq], RELU,
                                 bias=w_sb["b_enc1_4"][:], scale=1.0)

            # enc2: pA (pts 0,1), pB (pts 2,3) in separate psum tiles
            pe2a = ps.tile([128, 1024], F32, tag="ps")
            pe2b = ps.tile([128, 1024], F32, tag="ps")
            for s in range(0, cq, 512):
                w = min(512, cq - s)
                nc.tensor.matmul(pe2a[:, s:s + w], w_sb["enc2_lhsT"][:],
                                 h1[0:64, s:s + w], start=True, stop=True)
                nc.tensor.matmul(pe2b[:, s:s + w], enc2_hi[64:128, :],
                                 h1[64:128, s:s + w], start=True, stop=True)
            As = mid.tile([128, 1024], BF16, tag="As")
            nc.scalar.activation(As[:, :cq], pe2a[:, :cq], COPY)
            Cb = mid.tile([128, 1024], BF16, tag="Cb")
            nc.vector.tensor_max(Cb[:, :cq], pe2b[:, :cq], As[:, :cq])
            enc_tiles[k] = encT_t
            cb_tiles[k] = Cb

        def stage_B(k):
            """seg1 reduce -> neighT -> M -> per-chunk l1 lhsT."""
            (r0, r1, c0, cc) = plan["chunks"][k]
            cq = cc // 4
            cq2 = cq // 2
            nk = r1 - r0
            Cb = cb_tiles.pop(k)

            # level 0: max of the two chunk halves (same rank pairing)
            t1 = mid.tile([128, 512], BF16, tag="t1")
            nc.vector.tensor_max(t1[:, :cq2], Cb[:, 0:cq2], Cb[:, cq2:cq])
            # per class: one halving level then reduce
            for (i, j, hoff) in plan["classes"][k]:
                n = j - i
                w = int(L[i]) // 8          # quads per rank half
                q0 = hoff // 4
                a, odd = w // 2, w % 2
                wh = a + odd
                t2 = small.tile([128, 320], BF16, tag="t2")
                dst = t2[:, : n * wh].rearrange("p (n w) -> p n w", w=wh)
                src = t1[:, q0: q0 + n * w].rearrange("p (n w) -> p n w", w=w)
                nc.vector.tensor_max(dst[:, :, :a], src[:, :, :a],
                                     src[:, :, a:2 * a])
                if odd:
                    nc.vector.tensor_copy(dst[:, :, a:], src[:, :, 2 * a:])
                nc.vector.reduce_max(pre_neigh[:, i:j],
                                     t2[:, : n * wh].rearrange(
                                         "p (n w) -> p n w", w=wh),
                                     axis=mybir.AxisListType.X)

            # fold 128 partitions -> 64, relu+bias -> nT cols 64:64+nk
            fold = small.tile([64, 64], BF16, tag="fold")
            nc.gpsimd.dma_start(out=fold[:, :nk], in_=pre_neigh[64:128, r0:r1])
            mx = small.tile([64, 64], BF16, tag="mx")
            nc.vector.tensor_max(mx[:, :nk], pre_neigh[0:64, r0:r1],
                                 fold[:, :nk])
            nc.scalar.activation(nT[:, 64:64 + nk], mx[:, :nk], RELU,
                                 bias=w_sb["b_enc2"][:], scale=1.0)

            # M = nT^T @ W1c lands at psum partitions 64:64+nk; assemble
            # lhsT = [W1A(0:64) | M(64:125) | W1B(125:128)]
            pm = ps.tile([128, 1024], F32, tag="ps")
            nc.tensor.matmul(pm[:, 0:128], nT[:], w_sb["W1c"][:],
                             start=True, stop=True)
            lhsT_k = small.tile([128, 128], BF16, tag="lhsT")
            nc.gpsimd.dma_start(out=lhsT_k[0:64, :], in_=w_sb["W1A"][:])
            nc.gpsimd.dma_start(out=lhsT_k[125:128, :], in_=w_sb["W1B"][:])
            nc.scalar.activation(lhsT_k[64:125, :], pm[64:125, 0:128], COPY)
            lhsT_of[k] = lhsT_k

        def stage_C(k):
            """l1 + l2 + seg2 reduce -> T2 cols."""
            (r0, r1, c0, cc) = plan["chunks"][k]
            ch = cc // 2
            encT_t = enc_tiles.pop(k)
            lhsT_k = lhsT_of.pop(k)

            e1 = stream.tile([128, MAX_COLS], BF16, tag="e1", bufs=2)
            for t0 in range(0, cc, 1024):
                wcc = min(1024, cc - t0)
                p = ps.tile([128, 1024], F32, tag="ps")
                for s in range(0, wcc, 512):
                    w = min(512, wcc - s)
                    nc.tensor.matmul(p[:, s:s + w], lhsT_k[:],
                                     encT_t[:, t0 + s:t0 + s + w],
                                     start=True, stop=True)
                eng = L1_EVAC[l1_widx[0] % len(L1_EVAC)]
                l1_widx[0] += 1
                if eng == "S":
                    nc.scalar.activation(e1[:, t0:t0 + wcc], p[:, :wcc], RELU,
                                         bias=w_sb["b1"][:], scale=1.0)
                else:
                    nc.vector.tensor_scalar(
                        e1[:, t0:t0 + wcc], p[:, :wcc], w_sb["b1"][:], 0.0,
                        op0=mybir.AluOpType.add, op1=mybir.AluOpType.max,
                    )

            # l2 + pair max: pA over first half, pB over second half
            Db = mid.tile([128, 2048], BF16, tag="Db")
            for t0 in range(0, ch, 1024):
                wch = min(1024, ch - t0)
                pa = ps.tile([128, 1024], F32, tag="ps")
                pb = ps.tile([128, 1024], F32, tag="ps")
                for s in range(0, wch, 512):
                    w = min(512, wch - s)
                    nc.tensor.matmul(pa[:, s:s + w], w_sb["fcW2"][:],
                                     e1[:, t0 + s:t0 + s + w],
                                     start=True, stop=True)
                    nc.tensor.matmul(pb[:, s:s + w], w_sb["fcW2"][:],
                                     e1[:, ch + t0 + s:ch + t0 + s + w],
                                     start=True, stop=True)
                Ds = mid.tile([128, 1024], BF16, tag="Ds")
                nc.scalar.activation(Ds[:, :wch], pa[:, :wch], COPY)
                nc.vector.tensor_max(Db[:, t0:t0 + wch],
                                     pb[:, :wch], Ds[:, :wch])

            # seg2: per class one halving level then reduce -> T2
            for (i, j, hoff) in plan["classes"][k]:
                n = j - i
                w = int(L[i]) // 2          # cols per rank in Db
                a, odd = w // 2, w % 2
                wh = a + odd
                t3 = mid.tile([128, 1024], BF16, tag="t3")
                dst = t3[:, : n * wh].rearrange("p (n w) -> p n w", w=wh)
                src = Db[:, hoff: hoff + n * w].rearrange(
                    "p (n w) -> p n w", w=w)
                nc.vector.tensor_max(dst[:, :, :a], src[:, :, :a],
                                     src[:, :, a:2 * a])
                if odd:
                    nc.vector.tensor_copy(dst[:, :, a:], src[:, :, 2 * a:])
                nc.vector.reduce_max(T2buf[:, i:j],
                                     t3[:, : n * wh].rearrange(
                                         "p (n w) -> p n w", w=wh),
                                     axis=mybir.AxisListType.X)

        # ---- software pipeline: A(k+2) | B(k+1) | C(k)
        stage_A(0)
        stage_B(0)
        if n_chunks > 1:
            stage_A(1)
        for k in range(n_chunks):
            if k + 2 < n_chunks:
                stage_A(k + 2)
            if k + 1 < n_chunks:
                stage_B(k + 1)
            stage_C(k)

        # ---- global MLP
        gT = glob.tile([128, n_ranks], BF16, tag="gT")
        nc.scalar.activation(gT[:], T2buf[:], RELU, bias=w_sb["b2"][:],
                             scale=1.0)
        pg = ps.tile([128, 1024], F32, tag="ps")
        nc.tensor.matmul(pg[:, 0:512], w_sb["G1"][:], gT[:],
                         start=True, stop=True)
        g1T = glob.tile([128, n_ranks], BF16, tag="g1T")
        nc.scalar.activation(g1T[:], pg[:, 0:512], RELU,
                             bias=w_sb["gb1"][:], scale=1.0)
        po = ps.tile([128, 1024], F32, tag="ps")
        for half, (wn, bn) in enumerate((("G2a", "gb2a"), ("G2b", "gb2b"))):
            nc.tensor.matmul(po[:, half * 512:(half + 1) * 512],
                             w_sb[wn][:], g1T[:], start=True, stop=True)
        for half, (wn, bn) in enumerate((("G2a", "gb2a"), ("G2b", "gb2b"))):
            o_sb = glob.tile([128, 512], F32, tag=f"osb{half}")
            nc.scalar.activation(o_sb[:], po[:, half * 512:(half + 1) * 512],
                                 RELU, bias=w_sb[bn][:], scale=1.0)
            nc.sync.dma_start(out=out_d[half * 128:(half + 1) * 128, :],
                              in_=o_sb[:])

    nc.finalize()
    return nc


# ---------------------------------------------------------------- entry

_CACHE = {}


def _run(inputs, trace=False, **spmd_kwargs):
    cluster = np.asarray(inputs["cluster"])
    key = hash(cluster.tobytes())
    if key not in _CACHE:
        plan = _plan(cluster)
        nc = _build(plan)
        _CACHE[key] = (plan, nc)
    plan, nc = _CACHE[key]

    rel_points = np.asarray(inputs["relative_points"], dtype=np.float32)
    features = np.asarray(inputs["features"], dtype=np.float32)
    sort_idx = np.argsort(cluster, kind="stable")
    bucket0 = np.concatenate(
        [[0], np.cumsum(np.bincount(cluster, minlength=N_CLUSTERS))]
    )
    wmap = _prep_weights({k: np.asarray(v, dtype=np.float32)
                          for k, v in inputs.items()
                          if k not in ("relative_points", "features", "cluster")})

    in_maps = []
    for k in range(N_CORES):
        m = _prep_core(k, plan, rel_points, features, cluster, sort_idx, bucket0)
        m.update(wmap)
        in_maps.append(m)

    res = run_bass_kernel_spmd(nc, in_maps, list(range(N_CORES)),
                               trace=trace, **spmd_kwargs)

    out = np.empty((N_CLUSTERS, 256), dtype=np.float32)
    for k in range(N_CORES):
        out[plan["cids"][k]] = res.results[k]["out"].T
    return out, res


def kernel(**inputs):
    return _run(inputs)[0]


# revision 12
# speedup vs baseline: 1.6846x; 1.1126x over previous
"""Trainium2 Bass kernel for nn_MiddleLayerEncoder (gnn_message_passing).

Strategy: shard by CLUSTER across the 8 cores (each core owns 512 whole
clusters and all their points), so both segment_max reductions are
core-local and no collectives are needed.

v3 design: points sorted by cluster, padded per-rank to canonical size
L (multiple of 4, min 36, identical across cores -> one SPMD program),
grouped into chunks of <= 61 ranks / <= 4096 columns.  Both segment_max
stages are fused into PSUM evacuation: vector reduce_max with a 3D
access pattern reads matmul outputs straight out of PSUM per cluster
run -- no copies, no tree reductions, no intermediate buffers.  PSUM
windows are aligned to rank boundaries so each reduce covers whole
clusters.  The neigh_enc[cluster] gather is fused into the W1 matmul
via per-chunk one-hot rows (rank-in-chunk at partitions 64:125), so a
single per-chunk lhsT = [W1A | M | W1B] serves every l1 matmul of the
chunk; M = neighT^T @ W1c is placed at PSUM partitions 64:125 by the
matmul itself and evacuated in place.
"""

import numpy as np
import ml_dtypes
from contextlib import ExitStack

import concourse.bass as bass
import concourse.bacc as bacc
import concourse.tile as tile
from concourse import mybir
from concourse.bass_utils import run_bass_kernel_spmd

BF16 = mybir.dt.bfloat16
F32 = mybir.dt.float32
NPBF16 = ml_dtypes.bfloat16

N_CORES = 8
N_PTS = 262144
N_CLUSTERS = 4096
MINL = 36          # minimum padded points per cluster (multiple of 4)
MAX_RANKS = 61     # chunk rank cap: 64 + 61 + 3 = 128 lhsT partitions
MAX_COLS = 4096    # chunk column cap (SBUF tiling)
WIN = 1024         # PSUM window (2 banks)

# l1 evacuation engine per window, round-robin (S=scalar, V=vector)
L1_EVAC = "SSSV"

# packed bf16 weight slots: name -> (col0, rows)
WSLOT = {
    "enc1_lhsT": (0, (0, 12)),
    "enc2_lhsT": (128, (0, 64)),
    "enc2_hi": (256, (64, 128)),
    "W1A": (384, (0, 64)),
    "W1B": (512, (125, 128)),
    "W1c": (640, (0, 64)),
    "fcW2": (768, (0, 128)),
    "G1": (896, (0, 128)),
    "G2a": (1024, (0, 128)),
    "G2b": (1152, (0, 128)),
}
WCOLS = 1280
# packed f32 bias slots: name -> col
BSLOT = {"b_enc1_4": 0, "b_enc2": 1, "b1": 2, "b2": 3,
         "gb1": 4, "gb2a": 5, "gb2b": 6}
BCOLS = 7


# ---------------------------------------------------------------- planning

def _plan(cluster):
    """Canonical SPMD layout shared by all cores."""
    counts = np.bincount(cluster, minlength=N_CLUSTERS)
    assert counts.min() >= 1, "empty cluster unsupported"
    order = np.argsort(-counts, kind="stable")  # cluster ids, size desc

    # snake-deal into N_CORES bins -> per-core 512 clusters, balanced sizes
    n_ranks = N_CLUSTERS // N_CORES
    cids = np.empty((N_CORES, n_ranks), dtype=np.int64)
    for i, cid in enumerate(order):
        rnd, pos = divmod(i, N_CORES)
        core = pos if rnd % 2 == 0 else N_CORES - 1 - pos
        cids[core, rnd] = cid

    sizes = counts[cids]                      # [cores, ranks]
    L = sizes.max(axis=0)                     # canonical per-rank size
    L = np.maximum((L + 3) // 4 * 4, MINL).astype(np.int64)

    col0 = np.concatenate([[0], np.cumsum(L)])  # rank -> start col
    S = int(col0[-1])

    def runs(r0, r1):
        """class runs [(i, j)] of equal L within ranks [r0, r1)."""
        out = []
        i = r0
        while i < r1:
            j = i
            while j < r1 and L[j] == L[i]:
                j += 1
            out.append((i, j))
            i = j
        return out

    # chunks: whole clusters, <= MAX_COLS cols, <= MAX_RANKS ranks
    chunks = []  # dict per chunk
    r0 = 0
    while r0 < n_ranks:
        r1 = r0
        while (
            r1 < n_ranks
            and (col0[r1 + 1] - col0[r0]) <= MAX_COLS
            and (r1 - r0) < MAX_RANKS
        ):
            r1 += 1
        c0, cc = int(col0[r0]), int(col0[r1] - col0[r0])
        # rank-aligned PSUM windows of <= WIN cols
        wins = []
        i = r0
        while i < r1:
            j = i
            while j < r1 and col0[j + 1] - col0[i] <= WIN:
                j += 1
            wins.append(dict(
                i=i, j=j,
                off=int(col0[i] - col0[r0]),          # chunk-local col
                wcc=int(col0[j] - col0[i]),
                classes=[(a, b, int(col0[a] - col0[i])) for a, b in runs(i, j)],
            ))
            i = j
        chunks.append(dict(
            r0=r0, r1=r1, c0=c0, cc=cc, wins=wins,
            classes=[(a, b, int(col0[a] - col0[r0])) for a, b in runs(r0, r1)],
        ))
        r0 = r1

    return dict(cids=cids, sizes=sizes, L=L, col0=col0, S=S, chunks=chunks,
                n_ranks=n_ranks)


def _prep_core(k, plan, rel_points, features, cluster, sort_idx, bucket0):
    """Per-core input arrays (canonical layout, core-specific data)."""
    L, col0, S = plan["L"], plan["col0"], plan["S"]
    cids = plan["cids"][k]
    n_ranks = plan["n_ranks"]

    slot = np.empty(S, dtype=np.int64)
    oh_row = np.empty(S, dtype=np.int64)   # one-hot row: 64 + rank-in-chunk
    for ck in plan["chunks"]:
        for r in range(ck["r0"], ck["r1"]):
            cid = cids[r]
            idx = sort_idx[bucket0[cid]: bucket0[cid + 1]]
            n = idx.shape[0]
            c0, c1 = int(col0[r]), int(col0[r + 1])
            slot[c0: c0 + n] = idx
            if c1 - c0 > n:
                slot[c0 + n: c1] = idx[0]
            oh_row[c0:c1] = 64 + (r - ck["r0"])

    pts = rel_points[slot]          # [S, 3] f32
    feat = features[slot]           # [S, 64] f32

    # rows 0:64 = pts + feat[0:61] (= W1[0:64] order)
    # rows 64:125 = one-hot rank-in-chunk; rows 125:128 = feat[61:64]
    encT = np.zeros((128, S), dtype=NPBF16)
    encT[0:3] = pts.T.astype(NPBF16)
    encT[3:64] = feat.T[0:61].astype(NPBF16)
    encT[125:128] = feat.T[61:64].astype(NPBF16)
    encT[oh_row, np.arange(S)] = NPBF16(1.0)

    # pts4: [12, S/4] quad-packed points
    pts4 = (
        pts.astype(NPBF16)
        .reshape(S // 4, 4, 3)
        .transpose(1, 2, 0)
        .reshape(12, S // 4)
    )
    pts4 = np.ascontiguousarray(pts4)
    return {"encT": encT, "pts4": pts4}


def _blockdiag(w, times):
    fi, fo = w.shape
    out = np.zeros((fi * times, fo * times), dtype=w.dtype)
    for i in range(times):
        out[i * fi:(i + 1) * fi, i * fo:(i + 1) * fo] = w
    return out


def _prep_weights(inp):
    W1 = inp["W1"]
    mats = {
        "enc1_lhsT": _blockdiag(inp["enc_W1"], 4),
        "enc2_lhsT": _blockdiag(inp["enc_W2"], 2),
        "enc2_hi": _blockdiag(inp["enc_W2"], 2),
        "W1A": W1[0:64],
        "W1B": W1[64:67],
        "W1c": W1[67:131],
        "fcW2": inp["W2"],
        "G1": inp["G1"],
        "G2a": inp["G2"][:, 0:128],
        "G2b": inp["G2"][:, 128:256],
    }
    wpack = np.zeros((128, WCOLS), dtype=NPBF16)
    for name, (c, (p0, p1)) in WSLOT.items():
        m = mats[name]
        assert m.shape[0] == p1 - p0, name
        wpack[p0:p1, c:c + m.shape[1]] = m.astype(NPBF16)

    bias = {
        "b_enc1_4": np.tile(inp["enc_b1"], 4),
        "b_enc2": np.pad(inp["enc_b2"], (0, 64)),
        "b1": inp["b1"],
        "b2": inp["b2"],
        "gb1": inp["gb1"],
        "gb2a": inp["gb2"][0:128],
        "gb2b": inp["gb2"][128:256],
    }
    bpack = np.zeros((128, BCOLS), dtype=np.float32)
    for name, c in BSLOT.items():
        bpack[:, c] = bias[name]
    return {"wpack": wpack, "bpack": bpack}


# ---------------------------------------------------------------- program

def _build(plan):
    S = plan["S"]
    L = plan["L"]
    n_ranks = plan["n_ranks"]
    chunks = plan["chunks"]
    n_chunks = len(chunks)
    nc = bacc.Bacc(None, target_bir_lowering=False, debug=True)

    encT_d = nc.dram_tensor("encT", [128, S], BF16, kind="ExternalInput")
    pts4_d = nc.dram_tensor("pts4", [12, S // 4], BF16, kind="ExternalInput")
    wpack_d = nc.dram_tensor("wpack", [128, WCOLS], BF16, kind="ExternalInput")
    bpack_d = nc.dram_tensor("bpack", [128, BCOLS], F32, kind="ExternalInput")
    out_d = nc.dram_tensor("out", [256, 512], F32, kind="ExternalOutput")

    RELU = mybir.ActivationFunctionType.Relu
    COPY = mybir.ActivationFunctionType.Copy
    MAX = mybir.AluOpType.max
    ADD = mybir.AluOpType.add
    AXX = mybir.AxisListType.X

    with tile.TileContext(nc) as tc, ExitStack() as ctx:
        consts = ctx.enter_context(tc.tile_pool(name="consts", bufs=1))
        glob = ctx.enter_context(tc.tile_pool(name="glob", bufs=1))
        stream = ctx.enter_context(tc.tile_pool(name="stream", bufs=3))
        mid = ctx.enter_context(tc.tile_pool(name="mid", bufs=2))
        small = ctx.enter_context(tc.tile_pool(name="small", bufs=3))
        ps = ctx.enter_context(tc.tile_pool(name="ps", bufs=4, space="PSUM"))

        wp = consts.tile([128, WCOLS], BF16, tag="wpack")
        bp = consts.tile([128, BCOLS], F32, tag="bpack")

        def W(name):
            c, (p0, p1) = WSLOT[name]
            return wp[p0:p1, c:c + 128]

        def B(name):
            return bp[:, BSLOT[name]:BSLOT[name] + 1]

        def B64(name):
            return bp[0:64, BSLOT[name]:BSLOT[name] + 1]

        pre_neigh = glob.tile([128, n_ranks], BF16, tag="pre_neigh")
        T2buf = glob.tile([128, n_ranks], BF16, tag="T2buf")
        nT = glob.tile([64, 128], BF16, tag="nT")
        lhsT_bufs = []
        for i in range(3):
            lb = glob.tile([128, 128], BF16, tag=f"lhsT{i}", name=f"lhsT{i}")
            lhsT_bufs.append(lb)

        enc_tiles = {}   # chunk -> encT_t
        lhsT_of = {}     # chunk -> lhsT buffer
        l1_widx = [0]

        def stage_A_dma(k):
            ck = chunks[k]
            c0, cc = ck["c0"], ck["cc"]
            encT_t = stream.tile([128, MAX_COLS], BF16, tag="encT_t")
            nc.sync.dma_start(out=encT_t[:, :cc], in_=encT_d[:, c0:c0 + cc])
            pts4_t = mid.tile([12, MAX_COLS // 4], BF16, tag="pts4_t")
            nc.gpsimd.dma_start(out=pts4_t[:, :cc // 4],
                                in_=pts4_d[:, c0 // 4: c0 // 4 + cc // 4])
            enc_tiles[k] = (encT_t, pts4_t)

        def stage_A(k):
            """Encoder MLP; seg1 fused into PSUM reduce."""
            ck = chunks[k]
            r0, r1, cc = ck["r0"], ck["r1"], ck["cc"]
            cq = cc // 4
            nk = r1 - r0
            (encT_t, pts4_t) = enc_tiles[k]

            # enc1: [12 -> 128] on quad-packed points
            pe1 = ps.tile([128, WIN], F32, tag="ps")
            for s in range(0, cq, 512):
                w = min(512, cq - s)
                nc.tensor.matmul(pe1[:, s:s + w], W("enc1_lhsT"),
                                 pts4_t[:, s:s + w], start=True, stop=True)
            h1 = mid.tile([128, WIN], BF16, tag="h1")
            nc.scalar.activation(h1[:, :cq], pe1[:, :cq], RELU,
                                 bias=B("b_enc1_4"), scale=1.0)

            # enc2: pA (pts 0,1), pB (pts 2,3); seg1 via direct PSUM reduce
            pe2a = ps.tile([128, WIN], F32, tag="ps")
            pe2b = ps.tile([128, WIN], F32, tag="ps")
            for s in range(0, cq, 512):
                w = min(512, cq - s)
                nc.tensor.matmul(pe2a[:, s:s + w], W("enc2_lhsT"),
                                 h1[0:64, s:s + w], start=True, stop=True)
                nc.tensor.matmul(pe2b[:, s:s + w], W("enc2_hi"),
                                 h1[64:128, s:s + w], start=True, stop=True)
            TA = small.tile([128, 64], BF16, tag="TA")
            TB = small.tile([128, 64], BF16, tag="TB")
            for (a, b, off) in ck["classes"]:
                n = b - a
                w4 = int(L[a]) // 4
                q0 = off // 4
                va = pe2a[:, q0: q0 + n * w4].rearrange("p (n w) -> p n w", w=w4)
                vb = pe2b[:, q0: q0 + n * w4].rearrange("p (n w) -> p n w", w=w4)
                nc.vector.reduce_max(TA[:, a - r0: b - r0], va, axis=AXX)
                nc.vector.reduce_max(TB[:, a - r0: b - r0], vb, axis=AXX)
            nc.vector.tensor_max(pre_neigh[:, r0:r1], TA[:, :nk], TB[:, :nk])
            # fold 128 -> 64 partitions (issued here; consumed in stage_B)
            fold = small.tile([64, 64], BF16, tag="fold")
            nc.sync.dma_start(out=fold[:, :nk], in_=pre_neigh[64:128, r0:r1])
            return fold

        def stage_B(k, fold):
            """neighT -> M -> per-chunk l1 lhsT."""
            ck = chunks[k]
            r0, r1 = ck["r0"], ck["r1"]
            nk = r1 - r0
            mx = small.tile([64, 64], BF16, tag="mx")
            nc.vector.tensor_max(mx[:, :nk], pre_neigh[0:64, r0:r1],
                                 fold[:, :nk])
            nc.scalar.activation(nT[:, 64:64 + nk], mx[:, :nk], RELU,
                                 bias=B64("b_enc2"), scale=1.0)
            pm = ps.tile([128, WIN], F32, tag="ps")
            nc.tensor.matmul(pm[:, 0:128], nT[:], W("W1c"),
                             start=True, stop=True)
            lhsT_k = lhsT_bufs[k % 3]
            nc.scalar.activation(lhsT_k[64:125, :], pm[64:125, 0:128], COPY)
            lhsT_of[k] = lhsT_k

        def stage_C(k):
            """l1 + l2; seg2 fused into PSUM reduce."""
            ck = chunks[k]
            cc = ck["cc"]
            (encT_t, _) = enc_tiles.pop(k)
            lhsT_k = lhsT_of.pop(k)

            e1 = stream.tile([128, MAX_COLS], BF16, tag="e1", bufs=2)
            for win in ck["wins"]:
                off, wcc = win["off"], win["wcc"]
                p = ps.tile([128, WIN], F32, tag="ps")
                for s in range(0, wcc, 512):
                    w = min(512, wcc - s)
                    nc.tensor.matmul(p[:, s:s + w], lhsT_k[:],
                                     encT_t[:, off + s:off + s + w],
                                     start=True, stop=True)
                eng = L1_EVAC[l1_widx[0] % len(L1_EVAC)]
                l1_widx[0] += 1
                if eng == "S":
                    nc.scalar.activation(e1[:, off:off + wcc], p[:, :wcc],
                                         RELU, bias=B("b1"), scale=1.0)
                else:
                    nc.vector.tensor_scalar(
                        e1[:, off:off + wcc], p[:, :wcc], B("b1"), 0.0,
                        op0=ADD, op1=MAX,
                    )

            for win in ck["wins"]:
                off, wcc = win["off"], win["wcc"]
                p = ps.tile([128, WIN], F32, tag="ps")
                for s in range(0, wcc, 512):
                    w = min(512, wcc - s)
                    nc.tensor.matmul(p[:, s:s + w], W("fcW2"),
                                     e1[:, off + s:off + s + w],
                                     start=True, stop=True)
                for (a, b, woff) in win["classes"]:
                    n = b - a
                    wL = int(L[a])
                    v = p[:, woff: woff + n * wL].rearrange(
                        "p (n w) -> p n w", w=wL)
                    nc.vector.reduce_max(T2buf[:, a:b], v, axis=AXX)

        # ---- pipeline: prefetch DMA, then A(k+2) | B(k+1) | C(k)
        stage_A_dma(0)
        nc.sync.dma_start(out=wp[:], in_=wpack_d[:])
        nc.sync.dma_start(out=bp[:], in_=bpack_d[:])
        nc.vector.memset(nT[:], 0.0)
        for i, lb in enumerate(lhsT_bufs):
            nc.gpsimd.dma_start(out=lb[0:64, :],
                                in_=wpack_d[0:64, WSLOT["W1A"][0]:
                                            WSLOT["W1A"][0] + 128])
            nc.gpsimd.dma_start(out=lb[125:128, :],
                                in_=wpack_d[125:128, WSLOT["W1B"][0]:
                                            WSLOT["W1B"][0] + 128])
        if n_chunks > 1:
            stage_A_dma(1)
        fold0 = stage_A(0)
        stage_B(0, fold0)
        if n_chunks > 2:
            stage_A_dma(2)
        if n_chunks > 1:
            fold1 = stage_A(1)
        folds = {1: fold1} if n_chunks > 1 else {}
        for k in range(n_chunks):
            if k + 3 < n_chunks:
                stage_A_dma(k + 3)
            if k + 2 < n_chunks:
                folds[k + 2] = stage_A(k + 2)
            if k + 1 < n_chunks:
                stage_B(k + 1, folds.pop(k + 1))
            stage_C(k)

        # ---- global MLP
        gT = glob.tile([128, n_ranks], BF16, tag="gT")
        nc.scalar.activation(gT[:], T2buf[:], RELU, bias=B("b2"), scale=1.0)
        pg = ps.tile([128, WIN], F32, tag="ps")
        nc.tensor.matmul(pg[:, 0:512], W("G1"), gT[:], start=True, stop=True)
        g1T = glob.tile([128, n_ranks], BF16, tag="g1T")
        nc.scalar.activation(g1T[:], pg[:, 0:512], RELU,
                             bias=B("gb1"), scale=1.0)
        po = ps.tile([128, WIN], F32, tag="ps")
        nc.tensor.matmul(po[:, 0:512], W("G2a"), g1T[:], start=True, stop=True)
        nc.tensor.matmul(po[:, 512:1024], W("G2b"), g1T[:],
                         start=True, stop=True)
        for half, bn in enumerate(("gb2a", "gb2b")):
            o_sb = glob.tile([128, 512], F32, tag=f"osb{half}")
            nc.scalar.activation(o_sb[:], po[:, half * 512:(half + 1) * 512],
                                 RELU, bias=B(bn), scale=1.0)
            nc.sync.dma_start(out=out_d[half * 128:(half + 1) * 128, :],
                              in_=o_sb[:])

    nc.finalize()
    return nc


# ---------------------------------------------------------------- entry

_CACHE = {}


def _run(inputs, trace=False, **spmd_kwargs):
    cluster = np.asarray(inputs["cluster"])
    key = hash(cluster.tobytes())
    if key not in _CACHE:
        plan = _plan(cluster)
        nc = _build(plan)
        _CACHE[key] = (plan, nc)
    plan, nc = _CACHE[key]

    rel_points = np.asarray(inputs["relative_points"], dtype=np.float32)
    features = np.asarray(inputs["features"], dtype=np.float32)
    sort_idx = np.argsort(cluster, kind="stable")
    bucket0 = np.concatenate(
        [[0], np.cumsum(np.bincount(cluster, minlength=N_CLUSTERS))]
    )
    wmap = _prep_weights({k: np.asarray(v, dtype=np.float32)
                          for k, v in inputs.items()
                          if k not in ("relative_points", "features", "cluster")})

    in_maps = []
    for k in range(N_CORES):
        m = _prep_core(k, plan, rel_points, features, cluster, sort_idx, bucket0)
        m.update(wmap)
        in_maps.append(m)

    res = run_bass_kernel_spmd(nc, in_maps, list(range(N_CORES)),
                               trace=trace, **spmd_kwargs)

    out = np.empty((N_CLUSTERS, 256), dtype=np.float32)
    for k in range(N_CORES):
        out[plan["cids"][k]] = res.results[k]["out"].T
    return out, res


def kernel(**inputs):
    return _run(inputs)[0]
